# revision 9
# baseline (speedup 1.0000x reference)
"""Trainium2 Bass kernel for nn_DifferentiableLengthRegulator.

Reference computation (per batch b):
    cum = cumsum(durations)                         # [L]
    logits[t, l] = -|t + 0.5 - cum[l]| / 0.1        # [T, L], -inf on padding
    w = softmax(logits, axis=l)
    out[t, :] = sum_l w[t, l] * features[l, :]      # [T, D]

Device strategy (SPMD, 8 cores, 2 batches/core):
  The softmax weight for token l at frame t is exp(-10*|t+0.5-cum[l]|)
  (normalized); it decays by e^-10 per frame of distance, so only tokens
  whose cumulative end-time lies within +-9 frames of a chunk of frames can
  matter.  The host computes, per (batch, 256-frame chunk), a W-token
  window (W=128 typically) plus per-token scalars
      s1 = cum - t0 - 0.5           (frame-center offset)
      c  = cum_last + 6.5 - cum     (far-frame clamp, = BIG on padding)
  On device, with tokens on partitions and frames on the free axis:
      adist = |f + s1n|            (ACT Abs, per-partition bias s1n, f = iota)
      e0    = exp(-10 * adist)     (ACT Exp -> bf16)
      e     = max(e0, eta)         (DVE, eta = exp(-10*c) host-precomputed)
  max(e0, eta) == exp(-10*min(adist, c)); the clamp c reproduces the
  reference's softmax for frames beyond the sequence end exactly: softmax is
  shift-invariant, and for t past cum_last all distances share the same
  t-dependent offset.
  A ones-column appended to the features makes the same matmul that computes
  e.T @ features also produce the softmax denominator s; the PSUM->SBUF copy
  is fused with the 1/s scale.
"""

import os
import sys

sys.path.insert(0, '/opt/trn_rl_repo')
_HERE = os.path.dirname(os.path.abspath(__file__))
if _HERE not in sys.path:
    sys.path.insert(0, _HERE)

import numpy as np
import ml_dtypes

import concourse.bass as bass
import concourse.tile as tile
from concourse import mybir
from concourse.bass_utils import run_bass_kernel_spmd

def split_multi_waits(nc, max_waits=1):
    """The walrus build here accepts at most ONE sem-wait per instruction
    ("Too many sync wait commands" otherwise).  Tile attaches several waits
    to one instruction; since each engine executes its stream in order, an
    instruction with N waits is equivalent to N-1 single-wait NOPs on the
    same engine immediately before it."""
    nfixed = 0
    for fn in nc.m.functions:
        stack = list(getattr(fn, 'blocks', []) or [])
        seen = []
        while stack:
            bb = stack.pop()
            seen.append(bb)
            for sub in getattr(bb, 'blocks', []) or []:
                stack.append(sub)
        for bb in seen:
            insts = bb.instructions
            i = 0
            while i < len(insts):
                inst = insts[i]
                si = getattr(inst, 'sync_info', None)
                if si is not None and si.on_wait and len(si.on_wait) > max_waits:
                    waits = list(si.on_wait)
                    keep = waits[-max_waits:]
                    extra = waits[:-max_waits]
                    nops = []
                    for j in range(0, len(extra), max_waits):
                        nops.append(mybir.InstNoOp(
                            name=nc.get_next_instruction_name(),
                            engine=inst.engine, ins=[], outs=[],
                            sync_info=mybir.SyncInfo(
                                on_wait=extra[j:j + max_waits], on_update=[])))
                    inst.sync_info = mybir.SyncInfo(
                        on_wait=keep, on_update=list(si.on_update))
                    insts[i:i] = nops
                    i += len(nops)
                    nfixed += 1
                i += 1
    return nfixed


def _light_drain_and_barrier(self, tick_clock, wait_clock):
    """Cheaper TileContext tail.  The stock tail (drain + dense all-engine
    barrier + per-sem resets + second barrier) measures ~9us.  Equivalent
    sequencing: GPSIMD waits for every processor's final tick (split into
    single-wait NOPs for this walrus), then resets DMA state and range-clears
    the tile semaphores; a sem-only barrier keeps the other engines from
    ending before the clear."""
    from concourse.vector_clock import ScopedClock
    nc = self.nc
    probe = nc.gpsimd.nop(nofuse=True)
    wait_clock.add_sem_waits(probe.ins, ScopedClock({None: tick_clock.global_clock}))
    si = probe.ins.sync_info
    if si is not None and si.on_wait and len(si.on_wait) > 1:
        waits = list(si.on_wait)
        probe.ins.sync_info = mybir.SyncInfo(on_wait=waits[:1], on_update=[])
        for k in range(1, len(waits)):
            extra = nc.gpsimd.nop(nofuse=True)
            extra.ins.sync_info = mybir.SyncInfo(on_wait=waits[k:k + 1], on_update=[])
    nc.sync.drain()
    assert self.sems is not None
    popped = nc._tile_sem_poison_stack.pop()
    assert popped is self._sem_poison
    nc.clear_and_free_semaphores(list(self.sems.allocated().values()))
    # No trailing all-engine barrier: every engine's final tick was awaited
    # above before the clear, trailing per-engine DRAINs touch no bass sems,
    # and NRT serializes executions, so the next execution's preamble cannot
    # observe pre-clear semaphore state.


tile.TileContext._drain_and_barrier = _light_drain_and_barrier

A = mybir.AluOpType
F = mybir.ActivationFunctionType

B, L, D = 16, 512, 384
NCORES = 8
BPC = B // NCORES          # batches per core
CHUNK = 256                # frames per chunk (2 PSUM t-subtiles of 128)
MARGIN = 9.0               # window margin in frames (weight <= e^-85 outside)
CLAMP_OFF = 6.5            # far-frame clamp offset past cum_last
BIG = float(2 ** 30)       # "masked" sentinel

_BUILD_CACHE = {}
LAST_RESULTS = None        # BassKernelResults of the most recent run


def _build(NCH, NL):
    """Build the SPMD Bass program for NCH chunks of CHUNK frames and
    NL 128-token window tiles."""
    W = NL * 128
    Tpad = NCH * CHUNK
    XS = CHUNK // 128      # t-subtiles per chunk

    nc = bass.Bass("TRN2", num_devices=NCORES)
    fwin = nc.declare_dram_parameter(
        "fwin", [BPC, NCH, NL, 128, D + 1], mybir.dt.bfloat16, isOutput=False)
    scal = nc.declare_dram_parameter(
        "scal", [BPC, NCH, NL, 128, 2], mybir.dt.float32, isOutput=False)
    out = nc.declare_dram_parameter(
        "out", [BPC, Tpad, D], mybir.dt.float32, isOutput=True)

    # out-scale copies (PSUM->SBUF) go mostly to DVE; ACT takes every 4th
    # to balance (ACT already runs Abs+Exp per l-tile).
    def outscale_on_act(i):
        return i % 4 == 3

    with tile.TileContext(nc) as tc:
        with (
            tc.tile_pool(name="singles", bufs=1) as singles,
            tc.tile_pool(name="work", bufs=4) as work,
            tc.tile_pool(name="ework", bufs=4) as ework,
            tc.tile_pool(name="outp", bufs=4) as outp,
            tc.tile_pool(name="small", bufs=8) as small,
            tc.tile_pool(name="psum", bufs=4, space="PSUM") as psump,
        ):
            # iota row: frame index f along the free axis, same on every partition
            iota_i = singles.tile([128, CHUNK], mybir.dt.int32, tag="ii")
            nc.gpsimd.iota(iota_i, pattern=[[1, CHUNK]], base=0,
                           channel_multiplier=0)
            iota_f = singles.tile([128, CHUNK], mybir.dt.float32, tag="if")
            nc.vector.tensor_copy(iota_f, iota_i)

            # scalars first (small; they gate the first Abs), then features
            # split per (batch, chunk-half) so early chunks' matmuls don't
            # wait on the whole feature load.
            scal_sb = singles.tile([128, BPC, NCH, NL, 2],
                                   mybir.dt.float32, tag="sc")
            for j in range(BPC):
                nc.sync.dma_start(
                    out=scal_sb[:, j], in_=scal[j].rearrange("c l p s -> p c l s"))
            NH = max(1, NCH // 2)
            nhalves = (NCH + NH - 1) // NH
            fwin_tiles = {}
            for j in range(BPC):
                for h in range(nhalves):
                    c0 = h * NH
                    csz = min(NH, NCH - c0)
                    ft = singles.tile([128, csz, NL, D + 1],
                                      mybir.dt.bfloat16, tag=f"fw{j}_{h}")
                    fwin_tiles[(j, h)] = ft
                    nc.sync.dma_start(
                        out=ft,
                        in_=fwin[j, c0:c0 + csz].rearrange("c l p d -> p c l d"))

            def fwin_ap(j, c, li):
                h = c // NH
                return fwin_tiles[(j, h)][:, c - h * NH, li, :]

            out_r = out.rearrange("b (c x p) d -> b c p x d", x=XS, p=128)

            osc = 0
            for j in range(BPC):
                for c in range(NCH):
                    e_t = ework.tile([128, NL, CHUNK], mybir.dt.bfloat16, tag="e")
                    for li in range(NL):
                        s1n = scal_sb[:, j, c, li, 0:1]
                        eta = scal_sb[:, j, c, li, 1:2]
                        ad_t = work.tile([128, CHUNK], mybir.dt.float32, tag="ad")
                        nc.scalar.activation(
                            ad_t, iota_f, F.Abs, bias=s1n, scale=1.0)
                        nc.scalar.activation(
                            e_t[:, li, :], ad_t, F.Exp, scale=-10.0)
                        nc.vector.tensor_scalar(
                            e_t[:, li, :], e_t[:, li, :], scalar1=eta,
                            scalar2=None, op0=A.max)

                    psum_t = psump.tile([128, XS * 512], mybir.dt.float32,
                                        tag="ps")
                    for x in range(XS):
                        for li in range(NL):
                            nc.tensor.matmul(
                                psum_t[:, x * 512: x * 512 + D + 1],
                                lhsT=e_t[:, li, x * 128:(x + 1) * 128],
                                rhs=fwin_ap(j, c, li),
                                start=(li == 0), stop=(li == NL - 1))

                    rec = small.tile([128, XS, 1], mybir.dt.float32, tag="r")
                    nc.vector.reciprocal(
                        rec,
                        psum_t.rearrange("p (x n) -> p x n", n=512)[:, :, D:D + 1])

                    ot = outp.tile([128, XS, D], mybir.dt.float32, tag="o")
                    for x in range(XS):
                        src = psum_t[:, x * 512: x * 512 + D]
                        if outscale_on_act(osc):
                            nc.scalar.activation(
                                ot[:, x, :], src, F.Copy, bias=0.0,
                                scale=rec[:, x, :])
                        else:
                            nc.vector.tensor_scalar(
                                ot[:, x, :], src, scalar1=rec[:, x, :],
                                scalar2=None, op0=A.mult)
                        osc += 1
                    nc.sync.dma_start(out=out_r[j, c], in_=ot)

    split_multi_waits(nc)
    return nc


def _prepare(features, durations, padding_mask, total_frames):
    T = int(total_frames)
    f32 = np.float32
    cum = np.cumsum(durations.astype(f32), axis=1, dtype=f32)      # [B, L]
    valid = ~padding_mask
    nvalid = valid.sum(axis=1).astype(np.int64)                    # [B]
    cumlast = cum[np.arange(B), np.maximum(nvalid - 1, 0)]         # [B]

    NCH = max(1, (T + CHUNK - 1) // CHUNK)
    Tpad = NCH * CHUNK

    # per-(b, chunk) token windows
    los = np.zeros((B, NCH), np.int64)
    span_max = 1
    for b in range(B):
        nv = int(nvalid[b])
        cv = cum[b, :nv]
        for c in range(NCH):
            t0, t1 = c * CHUNK, (c + 1) * CHUNK
            lo = int(np.searchsorted(cv, t0 - MARGIN, 'left'))
            hi = int(np.searchsorted(cv, t1 + MARGIN, 'right'))
            if hi <= lo:        # chunk entirely past the sequence end
                lo, hi = max(0, nv - 1), nv
            los[b, c] = lo
            span_max = max(span_max, hi - lo)

    NL = min((span_max + 127) // 128, L // 128)
    W = NL * 128

    # gather windows
    fwin = np.zeros((B, NCH, W, D + 1), f32)
    scal = np.zeros((B, NCH, W, 2), f32)
    for b in range(B):
        nv = int(nvalid[b])
        for c in range(NCH):
            lo = int(min(max(los[b, c], 0), L - W))
            los[b, c] = lo
            t0 = c * CHUNK
            fwin[b, c, :, :D] = features[b, lo:lo + W, :]
            fwin[b, c, :, D] = 1.0
            cw = cum[b, lo:lo + W].astype(f32)
            tok_valid = (np.arange(lo, lo + W) < nv)
            # s1n: ACT Abs bias -> |iota + s1n| = |t + 0.5 - cum|
            s1n = np.where(tok_valid, f32(t0 + 0.5) - cw, f32(BIG))
            # eta = exp(-10 * clamp): far-frame floor, 0 for padded tokens
            cl = np.where(tok_valid,
                          cumlast[b] + f32(CLAMP_OFF) - cw, f32(np.inf))
            with np.errstate(under='ignore'):
                eta = np.exp(f32(-10.0) * cl.astype(np.float64)).astype(f32)
            scal[b, c, :, 0] = s1n
            scal[b, c, :, 1] = eta

    fwin = fwin.reshape(B, NCH, NL, 128, D + 1).astype(ml_dtypes.bfloat16)
    scal = scal.reshape(B, NCH, NL, 128, 2)
    return fwin, scal, T, Tpad, NCH, NL


def kernel(features, durations, padding_mask, total_frames):
    global LAST_RESULTS
    features = np.asarray(features, np.float32)
    durations = np.asarray(durations, np.float32)
    padding_mask = np.asarray(padding_mask, bool)

    fwin, scal, T, Tpad, NCH, NL = _prepare(
        features, durations, padding_mask, total_frames)

    key = (NCH, NL)
    if key not in _BUILD_CACHE:
        _BUILD_CACHE[key] = _build(NCH, NL)
    nc = _BUILD_CACHE[key]

    in_maps = []
    for core in range(NCORES):
        sl = slice(core * BPC, (core + 1) * BPC)
        in_maps.append({
            "fwin": np.ascontiguousarray(fwin[sl]),
            "scal": np.ascontiguousarray(scal[sl]),
        })

    res = run_bass_kernel_spmd(nc, in_maps, list(range(NCORES)))
    LAST_RESULTS = res

    out = np.empty((B, T, D), np.float32)
    for core in range(NCORES):
        out[core * BPC:(core + 1) * BPC] = res.results[core]["out"][:, :T, :]
    return out


# revision 15
# speedup vs baseline: 1.0029x; 1.0029x over previous
"""Trainium2 Bass kernel for nn_DifferentiableLengthRegulator.

Reference computation (per batch b):
    cum = cumsum(durations)                         # [L]
    logits[t, l] = -|t + 0.5 - cum[l]| / 0.1        # [T, L], -inf on padding
    w = softmax(logits, axis=l)
    out[t, :] = sum_l w[t, l] * features[l, :]      # [T, D]

Device strategy (SPMD, 8 cores, 2 batches/core):
  The softmax weight for token l at frame t is exp(-10*|t+0.5-cum[l]|)
  (normalized); it decays by e^-10 per frame of distance, so only tokens
  whose cumulative end-time lies within +-9 frames of a chunk of frames can
  matter.  The host computes, per (batch, 256-frame chunk), a W-token
  window (W=128 typically) plus per-token scalars
      s1 = cum - t0 - 0.5           (frame-center offset)
      c  = cum_last + 6.5 - cum     (far-frame clamp, = BIG on padding)
  On device, with tokens on partitions and frames on the free axis:
      adist = |f + s1n|            (ACT Abs, per-partition bias s1n, f = iota)
      e0    = exp(-10 * adist)     (ACT Exp -> bf16)
      e     = max(e0, eta)         (DVE, eta = exp(-10*c) host-precomputed)
  max(e0, eta) == exp(-10*min(adist, c)); the clamp c reproduces the
  reference's softmax for frames beyond the sequence end exactly: softmax is
  shift-invariant, and for t past cum_last all distances share the same
  t-dependent offset.
  A ones-column appended to the features makes the same matmul that computes
  e.T @ features also produce the softmax denominator s; the PSUM->SBUF copy
  is fused with the 1/s scale.
"""

import os
import sys

sys.path.insert(0, '/opt/trn_rl_repo')
_HERE = os.path.dirname(os.path.abspath(__file__))
if _HERE not in sys.path:
    sys.path.insert(0, _HERE)

import numpy as np
import ml_dtypes

import concourse.bass as bass
import concourse.tile as tile
from concourse import mybir
from concourse.bass_utils import run_bass_kernel_spmd

def split_multi_waits(nc, max_waits=1):
    """The walrus build here accepts at most ONE sem-wait per instruction
    ("Too many sync wait commands" otherwise).  Tile attaches several waits
    to one instruction; since each engine executes its stream in order, an
    instruction with N waits is equivalent to N-1 single-wait NOPs on the
    same engine immediately before it."""
    nfixed = 0
    for fn in nc.m.functions:
        stack = list(getattr(fn, 'blocks', []) or [])
        seen = []
        while stack:
            bb = stack.pop()
            seen.append(bb)
            for sub in getattr(bb, 'blocks', []) or []:
                stack.append(sub)
        for bb in seen:
            insts = bb.instructions
            i = 0
            while i < len(insts):
                inst = insts[i]
                si = getattr(inst, 'sync_info', None)
                if si is not None and si.on_wait and len(si.on_wait) > max_waits:
                    waits = list(si.on_wait)
                    keep = waits[-max_waits:]
                    extra = waits[:-max_waits]
                    nops = []
                    for j in range(0, len(extra), max_waits):
                        nops.append(mybir.InstNoOp(
                            name=nc.get_next_instruction_name(),
                            engine=inst.engine, ins=[], outs=[],
                            sync_info=mybir.SyncInfo(
                                on_wait=extra[j:j + max_waits], on_update=[])))
                    inst.sync_info = mybir.SyncInfo(
                        on_wait=keep, on_update=list(si.on_update))
                    insts[i:i] = nops
                    i += len(nops)
                    nfixed += 1
                i += 1
    return nfixed


def _light_drain_and_barrier(self, tick_clock, wait_clock):
    """Cheaper TileContext tail.  The stock tail (drain + dense all-engine
    barrier + per-sem resets + second barrier) measures ~9us.  Equivalent
    sequencing: GPSIMD waits for every processor's final tick (split into
    single-wait NOPs for this walrus), then resets DMA state and range-clears
    the tile semaphores; a sem-only barrier keeps the other engines from
    ending before the clear."""
    from concourse.vector_clock import ScopedClock
    nc = self.nc
    probe = nc.gpsimd.nop(nofuse=True)
    wait_clock.add_sem_waits(probe.ins, ScopedClock({None: tick_clock.global_clock}))
    si = probe.ins.sync_info
    if si is not None and si.on_wait and len(si.on_wait) > 1:
        waits = list(si.on_wait)
        probe.ins.sync_info = mybir.SyncInfo(on_wait=waits[:1], on_update=[])
        for k in range(1, len(waits)):
            extra = nc.gpsimd.nop(nofuse=True)
            extra.ins.sync_info = mybir.SyncInfo(on_wait=waits[k:k + 1], on_update=[])
    nc.sync.drain()
    assert self.sems is not None
    popped = nc._tile_sem_poison_stack.pop()
    assert popped is self._sem_poison
    nc.clear_and_free_semaphores(list(self.sems.allocated().values()))
    # No trailing all-engine barrier: every engine's final tick was awaited
    # above before the clear, trailing per-engine DRAINs touch no bass sems,
    # and NRT serializes executions, so the next execution's preamble cannot
    # observe pre-clear semaphore state.


tile.TileContext._drain_and_barrier = _light_drain_and_barrier

A = mybir.AluOpType
F = mybir.ActivationFunctionType

B, L, D = 16, 512, 384
NCORES = 8
BPC = B // NCORES          # batches per core
CHUNK = 256                # frames per chunk (2 PSUM t-subtiles of 128)
MARGIN = 9.0               # window margin in frames (weight <= e^-85 outside)
CLAMP_OFF = 6.5            # far-frame clamp offset past cum_last
BIG = float(2 ** 30)       # "masked" sentinel

_BUILD_CACHE = {}
LAST_RESULTS = None        # BassKernelResults of the most recent run


def _build(NCH, NL, need_clamp):
    """Build the SPMD Bass program for NCH chunks of CHUNK frames and
    NL 128-token window tiles.  need_clamp[c] marks chunks where some batch
    has frames past its sequence end (eta floor required)."""
    W = NL * 128
    Tpad = NCH * CHUNK
    XS = CHUNK // 128      # t-subtiles per chunk

    nc = bass.Bass("TRN2", num_devices=NCORES)
    fwin = nc.declare_dram_parameter(
        "fwin", [BPC, NCH, NL, 128, D + 1], mybir.dt.bfloat16, isOutput=False)
    scal = nc.declare_dram_parameter(
        "scal", [BPC, NCH, NL, 128, 2], mybir.dt.float32, isOutput=False)
    out = nc.declare_dram_parameter(
        "out", [BPC, Tpad, D], mybir.dt.bfloat16, isOutput=True)

    # out-scale copies (PSUM->SBUF) go mostly to DVE; ACT takes every 4th
    # to balance (ACT already runs Abs+Exp per l-tile).
    def outscale_on_act(i):
        return i % 4 == 3

    with tile.TileContext(nc) as tc:
        with (
            tc.tile_pool(name="singles", bufs=1) as singles,
            tc.tile_pool(name="work", bufs=4) as work,
            tc.tile_pool(name="ework", bufs=4) as ework,
            tc.tile_pool(name="outp", bufs=4) as outp,
            tc.tile_pool(name="small", bufs=8) as small,
            tc.tile_pool(name="psum", bufs=4, space="PSUM") as psump,
        ):
            # iota row: frame index f along the free axis, same on every partition
            iota_i = singles.tile([128, CHUNK], mybir.dt.int32, tag="ii")
            nc.gpsimd.iota(iota_i, pattern=[[1, CHUNK]], base=0,
                           channel_multiplier=0)
            iota_f = singles.tile([128, CHUNK], mybir.dt.float32, tag="if")
            nc.vector.tensor_copy(iota_f, iota_i)

            # scalars first (small; they gate the first Abs), then features
            # split per (batch, chunk-half) so early chunks' matmuls don't
            # wait on the whole feature load.
            scal_sb = singles.tile([128, BPC, NCH, NL, 2],
                                   mybir.dt.float32, tag="sc")
            for j in range(BPC):
                nc.sync.dma_start(
                    out=scal_sb[:, j], in_=scal[j].rearrange("c l p s -> p c l s"))
            NH = max(1, NCH // 2)
            nhalves = (NCH + NH - 1) // NH
            fwin_tiles = {}
            for j in range(BPC):
                for h in range(nhalves):
                    c0 = h * NH
                    csz = min(NH, NCH - c0)
                    ft = singles.tile([128, csz, NL, D + 1],
                                      mybir.dt.bfloat16, tag=f"fw{j}_{h}")
                    fwin_tiles[(j, h)] = ft
                    nc.sync.dma_start(
                        out=ft,
                        in_=fwin[j, c0:c0 + csz].rearrange("c l p d -> p c l d"))

            def fwin_ap(j, c, li):
                h = c // NH
                return fwin_tiles[(j, h)][:, c - h * NH, li, :]

            out_r = out.rearrange("b (c x p) d -> b c p x d", x=XS, p=128)

            osc = 0
            for j in range(BPC):
                for c in range(NCH):
                    e_t = ework.tile([128, NL, CHUNK], mybir.dt.bfloat16, tag="e")
                    for li in range(NL):
                        s1n = scal_sb[:, j, c, li, 0:1]
                        eta = scal_sb[:, j, c, li, 1:2]
                        ad_t = work.tile([128, CHUNK], mybir.dt.float32, tag="ad")
                        nc.scalar.activation(
                            ad_t, iota_f, F.Abs, bias=s1n, scale=1.0)
                        nc.scalar.activation(
                            e_t[:, li, :], ad_t, F.Exp, scale=-10.0)
                        if need_clamp[c]:
                            nc.vector.tensor_scalar(
                                e_t[:, li, :], e_t[:, li, :], scalar1=eta,
                                scalar2=None, op0=A.max)

                    psum_t = psump.tile([128, XS * 512], mybir.dt.float32,
                                        tag="ps")
                    for x in range(XS):
                        for li in range(NL):
                            nc.tensor.matmul(
                                psum_t[:, x * 512: x * 512 + D + 1],
                                lhsT=e_t[:, li, x * 128:(x + 1) * 128],
                                rhs=fwin_ap(j, c, li),
                                start=(li == 0), stop=(li == NL - 1))

                    rec = small.tile([128, XS, 1], mybir.dt.float32, tag="r")
                    nc.vector.reciprocal(
                        rec,
                        psum_t.rearrange("p (x n) -> p x n", n=512)[:, :, D:D + 1])

                    ot = outp.tile([128, XS, D], mybir.dt.bfloat16, tag="o")
                    for x in range(XS):
                        src = psum_t[:, x * 512: x * 512 + D]
                        if outscale_on_act(osc):
                            nc.scalar.activation(
                                ot[:, x, :], src, F.Copy, bias=0.0,
                                scale=rec[:, x, :])
                        else:
                            nc.vector.tensor_scalar(
                                ot[:, x, :], src, scalar1=rec[:, x, :],
                                scalar2=None, op0=A.mult)
                        osc += 1
                    nc.sync.dma_start(out=out_r[j, c], in_=ot)

    split_multi_waits(nc)
    return nc


def _prepare(features, durations, padding_mask, total_frames):
    T = int(total_frames)
    f32 = np.float32
    cum = np.cumsum(durations.astype(f32), axis=1, dtype=f32)      # [B, L]
    valid = ~padding_mask
    nvalid = valid.sum(axis=1).astype(np.int64)                    # [B]
    cumlast = cum[np.arange(B), np.maximum(nvalid - 1, 0)]         # [B]

    NCH = max(1, (T + CHUNK - 1) // CHUNK)
    Tpad = NCH * CHUNK

    # per-(b, chunk) token windows
    los = np.zeros((B, NCH), np.int64)
    span_max = 1
    for b in range(B):
        nv = int(nvalid[b])
        cv = cum[b, :nv]
        for c in range(NCH):
            t0, t1 = c * CHUNK, (c + 1) * CHUNK
            lo = int(np.searchsorted(cv, t0 - MARGIN, 'left'))
            hi = int(np.searchsorted(cv, t1 + MARGIN, 'right'))
            if hi <= lo:        # chunk entirely past the sequence end
                lo, hi = max(0, nv - 1), nv
            los[b, c] = lo
            span_max = max(span_max, hi - lo)

    NL = min((span_max + 127) // 128, L // 128)
    W = NL * 128

    # gather windows
    fwin = np.zeros((B, NCH, W, D + 1), f32)
    scal = np.zeros((B, NCH, W, 2), f32)
    for b in range(B):
        nv = int(nvalid[b])
        for c in range(NCH):
            lo = int(min(max(los[b, c], 0), L - W))
            los[b, c] = lo
            t0 = c * CHUNK
            fwin[b, c, :, :D] = features[b, lo:lo + W, :]
            fwin[b, c, :, D] = 1.0
            cw = cum[b, lo:lo + W].astype(f32)
            tok_valid = (np.arange(lo, lo + W) < nv)
            # s1n: ACT Abs bias -> |iota + s1n| = |t + 0.5 - cum|
            s1n = np.where(tok_valid, f32(t0 + 0.5) - cw, f32(BIG))
            # eta = exp(-10 * clamp): far-frame floor, 0 for padded tokens
            cl = np.where(tok_valid,
                          cumlast[b] + f32(CLAMP_OFF) - cw, f32(np.inf))
            with np.errstate(under='ignore'):
                eta = np.exp(f32(-10.0) * cl.astype(np.float64)).astype(f32)
            scal[b, c, :, 0] = s1n
            scal[b, c, :, 1] = eta

    need_clamp = tuple(bool((scal[:, c, :, 1] > 0).any()) for c in range(NCH))
    fwin = fwin.reshape(B, NCH, NL, 128, D + 1).astype(ml_dtypes.bfloat16)
    scal = scal.reshape(B, NCH, NL, 128, 2)
    return fwin, scal, T, Tpad, NCH, NL, need_clamp


def kernel(features, durations, padding_mask, total_frames):
    global LAST_RESULTS
    features = np.asarray(features, np.float32)
    durations = np.asarray(durations, np.float32)
    padding_mask = np.asarray(padding_mask, bool)

    fwin, scal, T, Tpad, NCH, NL, need_clamp = _prepare(
        features, durations, padding_mask, total_frames)

    key = (NCH, NL, need_clamp)
    if key not in _BUILD_CACHE:
        _BUILD_CACHE[key] = _build(NCH, NL, need_clamp)
    nc = _BUILD_CACHE[key]

    in_maps = []
    for core in range(NCORES):
        sl = slice(core * BPC, (core + 1) * BPC)
        in_maps.append({
            "fwin": np.ascontiguousarray(fwin[sl]),
            "scal": np.ascontiguousarray(scal[sl]),
        })

    res = run_bass_kernel_spmd(nc, in_maps, list(range(NCORES)))
    LAST_RESULTS = res

    out = np.empty((B, T, D), np.float32)
    for core in range(NCORES):
        out[core * BPC:(core + 1) * BPC] = \
            res.results[core]["out"][:, :T, :].astype(np.float32)
    return out
